# revision 18
# baseline (speedup 1.0000x reference)
"""Trainium2 Bass kernel for the dual channel-attention module.

Data-parallel over batch: B=8 -> one batch item per NeuronCore. Each core runs
two independent pipelines (y -> o1, x -> o2); each pipeline is:
  3x3 conv projections (Q,K stride 2) fused with BatchNorm,
  channel attention S = Q K^T (over tokens), softmax over channels,
  then the softmax matrices are folded INTO the V-conv weights on device:
    mean_h(P_h @ (W_vh (*) img)) = (sum_h P_h W_vh / H) (*) img
  so the per-head V conv (heads*C output channels) + per-head context matmuls
  collapse into one C-channel stride-1 conv, followed by out = ctx^T @ W_out^T.

All matmuls run as float32r (full PE rate at free-dim>=256, fp22 mantissa).
BN scale (and the attention 1/sqrt(C) for Q, and the 1/heads for the output
projection) are folded into weights on the host; BN bias is applied via a
ones-column bias matmul (Q/K, channel on free axis) or, for the fused V conv,
a per-partition activation bias computed on device from P_h and the V shifts.
"""

import os
import sys

sys.path.insert(0, '/opt/trn_rl_repo')

import numpy as np

import concourse.bacc as bacc
import concourse.mybir as mybir
import concourse.tile as tile
from concourse.bass_utils import run_bass_kernel_spmd
from concourse.masks import make_identity

F32 = mybir.dt.float32
F32R = mybir.dt.float32r
AF = mybir.ActivationFunctionType
AX = mybir.AxisListType

P = 128
C = 256          # channels
HEADS = 4
NCORES = 8
EPS = 1e-5

_programs = {}


def _build_program(H, W):
    """One-core program; same NEFF runs SPMD on all 8 cores."""
    N = H * W                 # stride-1 token count
    PH, PW = H + 2, W + 2     # padded image dims
    OH, OW = H // 2, W // 2   # stride-2 output dims
    NQ = OH * OW              # stride-2 token count
    T = NQ // P               # q/k token chunks
    RQ = P // OW              # stride-2 output rows per token chunk
    T2 = N // P               # input token chunks (and proj chunks)
    NT = N // 512             # v-conv tiles of 512 tokens
    RPN = 512 // W            # image rows per v tile
    CC = C // P               # channel chunks (2)

    nc = bacc.Bacc("TRN2", target_bir_lowering=False, debug=False,
                   num_devices=NCORES)

    # ---- I/O ----
    xin = [nc.dram_tensor(f"in{s}", [N, C], F32R, kind="ExternalInput").ap()
           for s in range(2)]
    wqk = nc.dram_tensor("wqk", [2, 2, HEADS // 2, CC, P, 9, 2 * C], F32R,
                         kind="ExternalInput").ap()
    # V weights pre-arranged for the on-device fold:
    # wvf[s, cich, tap, dP, h, dch, ciP] = Wv[conv_v(s)][h, dch*P+dP,
    #                                        cich*P+ciP, tap//3, tap%3]
    wvf = nc.dram_tensor("wvf", [2, CC, 9, P, HEADS, CC, P], F32R,
                         kind="ExternalInput").ap()
    bqk = nc.dram_tensor("bqk", [2, 2, P, HEADS, C], F32R,
                         kind="ExternalInput").ap()
    # V-conv BN shifts, replicated x8 on the last axis (fp32r matmuls
    # reject free-dim-1 moving operands)
    bv = nc.dram_tensor("bv", [2, HEADS, CC, P, 8], F32R,
                        kind="ExternalInput").ap()
    wo = nc.dram_tensor("wo", [2, CC, P, C], F32R, kind="ExternalInput").ap()
    outs = [nc.dram_tensor(f"out{s}", [N, C], F32, kind="ExternalOutput").ap()
            for s in range(2)]

    # tap decomposition for stride-2 grids: (dy,dx) -> grid (py,px,b) + row off a
    # grid combos (py, px, b): 6 of them
    combos = [(0, 0, 0), (0, 1, 0), (0, 0, 1), (1, 0, 0), (1, 1, 0), (1, 0, 1)]
    combo_idx = {c: i for i, c in enumerate(combos)}

    with tile.TileContext(nc, pool_alloc_mode="queue") as tc:
        import contextlib
        with contextlib.ExitStack() as est:
            consts = est.enter_context(tc.tile_pool(name="consts", bufs=1))
            sb_work = est.enter_context(tc.tile_pool(name="work", bufs=1))
            ps_tr = est.enter_context(
                tc.tile_pool(name="ps_tr", bufs=4, space="PSUM"))
            ps_ctx = est.enter_context(
                tc.tile_pool(name="ps_ctx", bufs=4, space="PSUM"))

            ident = consts.tile([P, P], F32)
            make_identity(nc, ident[:])
            ones_f = consts.tile([P, P], F32)
            nc.vector.memset(ones_f[:], 1.0)
            ones = consts.tile([P, P], F32R)
            nc.vector.tensor_copy(ones[:], ones_f[:])
            zeros_f = consts.tile([P, 2 * PW], F32)
            nc.vector.memset(zeros_f[:], 0.0)
            ident_r = consts.tile([P, P], F32R)
            nc.vector.tensor_copy(ident_r[:], ident[:])

            def phase_a(s, sb_img, hooks=None):
                """padded channel-major image via PE transposes

                hooks[t] is emitted right after chunk t's token DMA — used
                to enqueue weight-prefetch DMAs BEHIND the critical-path
                token loads (queues are FIFO; a big weight DMA emitted
                first would delay every token chunk behind it)."""
                img = [sb_img.tile([P, PH, PW], F32R, name=f"imgc{s}{cc}",
                                   tag=f"imgc{cc}") for cc in range(CC)]
                for cc in range(CC):
                    # zero borders: top+bottom rows, then left+right cols
                    nc.vector.tensor_copy(
                        img[cc][:, 0:PH:PH - 1, :], zeros_f[:, : 2 * PW]
                        .rearrange("p (a b) -> p a b", a=2))
                    nc.vector.tensor_copy(
                        img[cc][:, 1:PH - 1, 0:PW:PW - 1],
                        zeros_f[:, : 2 * H]
                        .rearrange("p (a b) -> p b a", a=2))
                for t in range(T2):
                    tok = sb_work.tile([P, C], F32R, name="tok", tag="tok",
                                       bufs=4)
                    nc.sync.dma_start(tok[:], xin[s][t * P:(t + 1) * P, :])
                    if hooks and t in hooks:
                        hooks[t]()
                    r0 = (t * P) // W
                    nr = P // W
                    for cc in range(CC):
                        ptp = ps_tr.tile([P, P], F32R, name="ptp", tag="pst")
                        nc.tensor.transpose(
                            ptp[:], tok[:, cc * P:(cc + 1) * P], ident_r[:])
                        nc.vector.tensor_copy(
                            img[cc][:, 1 + r0:1 + r0 + nr, 1:1 + W],
                            ptp[:].rearrange("p (a b) -> p a b", a=nr))
                return img

            def phase_b(s, img, sb_gr):
                """parity-compacted grids for stride-2 conv stationary tiles"""
                gr = [[sb_gr.tile([P, (OH + 1) * OW], F32R,
                                  name=f"g{s}{gi}_{cc}", tag=f"g{gi}_{cc}")
                       for cc in range(CC)] for gi in range(6)]
                uh = (OH + 1) // 2
                for gi, (py, px, b) in enumerate(combos):
                    c0 = 2 * b + px
                    for cc in range(CC):
                        for half, (u0, u1) in enumerate([(0, uh),
                                                         (uh, OH + 1)]):
                            dst = gr[gi][cc][:, u0 * OW:u1 * OW] \
                                .rearrange("p (u v) -> p u v", u=u1 - u0)
                            src = img[cc][:, py + 2 * u0: py + 2 * u1 - 1: 2,
                                          c0: c0 + 2 * OW - 1: 2]
                            if (gi + cc + half) % 2:
                                nc.vector.tensor_copy(dst, src)
                            else:
                                nc.scalar.copy(dst, src)
                return gr

            def load_biasb(s, sb_qk):
                biasb = [sb_qk.tile([P, HEADS, C], F32R, name=f"biasb{qk}",
                                    tag=f"biasb{qk}") for qk in range(2)]
                for qk in range(2):
                    nc.sync.dma_start(biasb[qk][:], bqk[s, qk])
                return biasb

            def load_qkw_one(s, sb_qkw, qk, pr, ci):
                wt = sb_qkw.tile([P, 9, 2 * C], F32R, name=f"wqk{qk}c{ci}",
                                 tag="qkw", bufs=3)
                nc.sync.dma_start(wt[:], wqk[s, qk, pr, ci])
                return wt

            def load_qkw(s, sb_qkw, qk, pr):
                return [load_qkw_one(s, sb_qkw, qk, pr, ci)
                        for ci in range(CC)]

            def phase_c(s, gr, sb_qkw, sb_qk, pT, biasb, pre_wt=None):
                """Q/K convs (stride 2, token-major) + channel attention.

                Returns a deferred closure emitting the last pair's softmax +
                p-transposes (so the caller can keep them off the PE critical
                path at the phase boundary)."""
                def softmax_block(pr, s_ps):
                    work_items = [(hl, ccb) for hl in range(2)
                                  for ccb in range(CC)]
                    negmax = {}
                    for hl, ccb in work_items:
                        nm = sb_work.tile([P, 1], F32, name="negmax",
                                          tag=f"negmax{hl}{ccb}")
                        nc.vector.reduce_max(nm[:], s_ps[hl][ccb][:],
                                             axis=AX.X, negate=True)
                        negmax[hl, ccb] = nm
                    e = {}
                    esum = {}
                    for hl, ccb in work_items:
                        ee = sb_work.tile([P, C], F32, name="esm",
                                          tag=f"esm{hl}{ccb}")
                        es = sb_work.tile([P, 1], F32, name="esum",
                                          tag=f"esum{hl}{ccb}")
                        nc.scalar.activation(ee[:], s_ps[hl][ccb][:], AF.Exp,
                                             bias=negmax[hl, ccb][:],
                                             scale=1.0, accum_out=es[:])
                        e[hl, ccb] = ee
                        esum[hl, ccb] = es
                    pn = {}
                    for hl, ccb in work_items:
                        rec = sb_work.tile([P, 1], F32, name="rec",
                                           tag=f"rec{hl}{ccb}")
                        nc.vector.reciprocal(rec[:], esum[hl, ccb][:])
                        pp = sb_work.tile([P, C], F32, name="pn",
                                          tag=f"pn{hl}{ccb}")
                        nc.vector.tensor_scalar_mul(pp[:], e[hl, ccb][:],
                                                    rec[:])
                        pn[hl, ccb] = pp
                    for hl, ccb in work_items:
                        h = 2 * pr + hl
                        for dc in range(CC):
                            ptp = ps_tr.tile([P, P], F32, name="ptp2",
                                             tag="pst")
                            nc.tensor.transpose(
                                ptp[:], pn[hl, ccb][:, dc * P:(dc + 1) * P],
                                ident[:])
                            nc.vector.tensor_copy(
                                pT[h][:, dc, ccb * P:(ccb + 1) * P],
                                ptp[:])

                deferred = None
                for pr in range(HEADS // 2):
                    s_ps = [[ps_ctx.tile([P, C], F32, name=f"sps{hl}{ccb}",
                                         tag="psc")
                             for ccb in range(CC)] for hl in range(2)]
                    qt_all = [sb_qk.tile([P, 2 * C], F32R, name=f"qt{t}",
                                         tag=f"qt{t}") for t in range(T)]
                    for qk in range(2):
                        if pr == 0 and qk == 0 and pre_wt is not None:
                            wt = pre_wt
                        else:
                            wt = load_qkw(s, sb_qkw, qk, pr)
                        for t in range(T):
                            acc = ps_tr.tile([P, 2 * C], F32, name="qkacc",
                                             tag="pst")
                            first = True
                            for ci in range(CC):
                                for tap in range(9):
                                    dy, dx = tap // 3, tap % 3
                                    gi = combo_idx[(dy & 1, dx & 1, dx >> 1)]
                                    a = dy >> 1
                                    off = (t * RQ + a) * OW
                                    nc.tensor.matmul(
                                        acc[:], gr[gi][ci][:, off:off + P],
                                        wt[ci][:, tap, :],
                                        start=first, stop=False)
                                    first = False
                            nc.tensor.matmul(
                                acc[:], ones[:, :P],
                                biasb[qk][:, 2 * pr:2 * pr + 2, :],
                                start=False, stop=True)
                            if qk == 0:
                                nc.scalar.copy(qt_all[t][:], acc[:])
                            else:
                                kt = sb_qk.tile([P, 2 * C], F32R, name="kt",
                                                tag="kt", bufs=3)
                                nc.scalar.copy(kt[:], acc[:])
                                for hl in range(2):
                                    for ccb in range(CC):
                                        nc.tensor.matmul(
                                            s_ps[hl][ccb][:],
                                            qt_all[t][:,
                                                      hl * C + ccb * P:
                                                      hl * C + (ccb + 1) * P],
                                            kt[:, hl * C:(hl + 1) * C],
                                            start=(t == 0),
                                            stop=(t == T - 1))
                        if qk == 0 and deferred is not None:
                            # previous pair's softmax+transposes, off the
                            # boundary critical path
                            deferred()
                            deferred = None
                    deferred = (lambda pr=pr, s_ps=s_ps:
                                softmax_block(pr, s_ps))
                return deferred

            def phase_d(s, img, pT, sb_vw, sb_wd, deferred=None):
                """Fold softmax into V weights, then one fused conv + proj.

                weff[ci, tap, c] = sum_{h,d} Wv_h[d, ci, tap] * P_h[c, d]
                cbias[c]         = sum_{h,d} P_h[c, d] * bshift_vh[d]
                ctx[c, n] = (weff (*) img)[c, n] + cbias[c]   (mean-over-heads
                folded into wo on host), out = ctx^T @ wo.
                """
                wot = [sb_wd.tile([P, C], F32R, name=f"wo{ccb}",
                                  tag=f"wo{ccb}") for ccb in range(CC)]
                for ccb in range(CC):
                    nc.sync.dma_start(wot[ccb][:], wo[s, ccb])
                bvt = [[sb_wd.tile([P, 8], F32R, name=f"bv{h}{dc}",
                                   tag=f"bvt{h}{dc}") for dc in range(CC)]
                       for h in range(HEADS)]
                for h in range(HEADS):
                    for dc in range(CC):
                        nc.sync.dma_start(bvt[h][dc][:], bv[s, h, dc])
                # stream fold-weight tiles; prime the pipeline before use
                blocks = [(ci, tp) for ci in range(CC) for tp in range(9)]
                wvq = []

                def push_wv(i):
                    cich, tap = blocks[i]
                    t = sb_vw.tile([P, HEADS, CC, P], F32R, name="wvt",
                                   tag="wvt", bufs=5)
                    nc.sync.dma_start(t[:], wvf[s, cich, tap])
                    wvq.append(t)

                for i in range(4):
                    push_wv(i)
                if deferred is not None:
                    deferred()   # last pair's softmax + pT transposes
                # ---- cbias via tiny matmuls: cb[c] = sum_h P_h[c,:] @ bv_h
                cb = []
                for cch in range(CC):
                    cps = ps_ctx.tile([P, 8], F32, name="cps", tag="psc")
                    first = True
                    for h in range(HEADS):
                        for dch in range(CC):
                            nc.tensor.matmul(
                                cps[:],
                                pT[h][:, dch, cch * P:(cch + 1) * P],
                                bvt[h][dch][:],
                                start=first,
                                stop=(h == HEADS - 1 and dch == CC - 1))
                            first = False
                    cbt = sb_wd.tile([P, 1], F32, name=f"cb{cch}",
                                     tag=f"cb{cch}")
                    nc.scalar.copy(cbt[:], cps[:, 0:1])
                    cb.append(cbt)
                # ---- weff fold: 18 blocks of 8 accumulating matmuls
                weff = [sb_wd.tile([P, 9, C], F32R, name=f"weff{ci}",
                                   tag=f"weff{ci}") for ci in range(CC)]
                for bi, (cich, tap) in enumerate(blocks):
                    wvt = wvq.pop(0)
                    wps = ps_tr.tile([P, C], F32, name="wps", tag="pst")
                    first = True
                    for h in range(HEADS):
                        for dch in range(CC):
                            nc.tensor.matmul(
                                wps[:], wvt[:, h, dch, :],
                                pT[h][:, dch, :],
                                start=first,
                                stop=(h == HEADS - 1 and dch == CC - 1))
                            first = False
                    nc.scalar.copy(weff[cich][:, tap, :], wps[:])
                    if bi + 4 < len(blocks):
                        push_wv(bi + 4)
                # ---- fused conv (stride 1) + output projection per ntile
                for nt in range(NT):
                    r0 = nt * RPN
                    vsb = []
                    for cch in range(CC):
                        facc = ps_tr.tile([P, 512], F32, name="facc",
                                          tag="pst")
                        first = True
                        for cich in range(CC):
                            for tap in range(9):
                                dy, dx = tap // 3, tap % 3
                                nc.tensor.matmul(
                                    facc[:],
                                    weff[cich][:, tap, cch * P:(cch + 1) * P],
                                    img[cich][:, r0 + dy: r0 + dy + RPN,
                                            dx:dx + W],
                                    start=first,
                                    stop=(cich == CC - 1 and tap == 8))
                                first = False
                        vt = sb_wd.tile([P, 512], F32R, name="vsb",
                                        tag="vsb", bufs=4)
                        nc.scalar.activation(vt[:], facc[:], AF.Identity,
                                             bias=cb[cch][:], scale=1.0)
                        vsb.append(vt)
                    for sub in range(512 // P):
                        t = nt * (512 // P) + sub
                        oacc = ps_ctx.tile([P, C], F32, name="oacc",
                                          tag="psc")
                        for cch in range(CC):
                            nc.tensor.matmul(
                                oacc[:],
                                vsb[cch][:, sub * P:(sub + 1) * P],
                                wot[cch][:],
                                start=(cch == 0), stop=(cch == CC - 1))
                        osb = sb_wd.tile([P, C], F32, name="osb",
                                         tag="osb", bufs=3)
                        nc.scalar.copy(osb[:], oacc[:])
                        nc.sync.dma_start(
                            outs[s][t * P:(t + 1) * P, :], osb[:])

            # ---- interleaved two-stream schedule ----
            # stream 0 (y): A, B+C; then D while stream 1's image builds.
            # Pool enter order is ring-allocation order (queue mode):
            # persistent img/keep first, then the per-stream C pools so the
            # whole C region frees as one block at each stream boundary.
            st0 = contextlib.ExitStack()
            sb_img0 = st0.enter_context(tc.tile_pool(name="img0", bufs=1))
            sb_keep0 = st0.enter_context(tc.tile_pool(name="keep0", bufs=1))
            cst = contextlib.ExitStack()
            sb_gr = cst.enter_context(tc.tile_pool(name="gr0", bufs=1,
                                                   side="right"))
            sb_qkw = cst.enter_context(tc.tile_pool(name="qkw0", bufs=1,
                                                    side="right"))
            sb_qk = cst.enter_context(tc.tile_pool(name="qk0", bufs=1,
                                                   side="right"))
            # first conv weights + biases prefetch INTERLEAVED into the
            # token stream: tokens are the critical path, weights have
            # ~10us of slack until the first Q-conv matmul needs them
            pre = {}
            hooks = {
                T2 // 2: lambda: pre.update(
                    w0=load_qkw_one(0, sb_qkw, 0, 0, 0)),
                T2 - 1: lambda: pre.update(
                    w1=load_qkw_one(0, sb_qkw, 0, 0, 1),
                    bb=load_biasb(0, sb_qk)),
            }
            img0 = phase_a(0, sb_img0, hooks)
            pre_wt0 = [pre['w0'], pre['w1']]
            biasb0 = pre['bb']
            pT0 = [sb_keep0.tile([P, CC, C], F32R, name=f"pT0{h}",
                                 tag=f"pT{h}") for h in range(HEADS)]
            gr0 = phase_b(0, img0, sb_gr)
            defer0 = phase_c(0, gr0, sb_qkw, sb_qk, pT0, biasb0,
                             pre_wt=pre_wt0)
            cst.close()

            d0 = contextlib.ExitStack()
            sb_vw0 = d0.enter_context(tc.tile_pool(name="vw0", bufs=1))
            sb_wd0 = d0.enter_context(tc.tile_pool(name="wd0", bufs=1))
            phase_d(0, img0, pT0, sb_vw0, sb_wd0, deferred=defer0)
            # stream 1 image: reuses stream-0 img/keep slots (tag reuse gives
            # precise deps on stream-0's last reads, no pool-boundary stalls)
            img1 = phase_a(1, sb_img0)
            pT1 = [sb_keep0.tile([P, CC, C], F32R, name=f"pT1{h}",
                                 tag=f"pT{h}") for h in range(HEADS)]
            d0.close()

            with contextlib.ExitStack() as cst1:
                # qkw1+qk1 place at the ring head; gr1 first-fits into the
                # hole left by vw0+wd0 (sized to fit it — see wvt bufs).
                sb_qkw = cst1.enter_context(tc.tile_pool(name="qkw1", bufs=1))
                sb_qk = cst1.enter_context(tc.tile_pool(name="qk1", bufs=1))
                sb_gr = cst1.enter_context(tc.tile_pool(name="gr1", bufs=1))
                biasb1 = load_biasb(1, sb_qk)
                gr1 = phase_b(1, img1, sb_gr)
                defer1 = phase_c(1, gr1, sb_qkw, sb_qk, pT1, biasb1)
            with contextlib.ExitStack() as dst_:
                sb_vw1 = dst_.enter_context(tc.tile_pool(name="vw1", bufs=1))
                sb_wd1 = dst_.enter_context(tc.tile_pool(name="wd1", bufs=1))
                phase_d(1, img1, pT1, sb_vw1, sb_wd1, deferred=defer1)
            st0.close()

    nc.compile()
    return nc


def _prep_weights(w_conv, bn_gamma, bn_beta, bn_mean, bn_var, w_out1, w_out2):
    """Fold BN into conv weights/biases and pack into kernel layouts."""
    w_conv = np.asarray(w_conv, np.float32)
    scale = np.asarray(bn_gamma, np.float32) / np.sqrt(
        np.asarray(bn_var, np.float32) + EPS)            # [6,4,256]
    shift = np.asarray(bn_beta, np.float32) - np.asarray(
        bn_mean, np.float32) * scale

    wf = w_conv * scale[:, :, :, None, None, None]       # [6,4,co,ci,3,3]
    sa = 1.0 / np.sqrt(C)
    wf[0] *= sa
    wf[1] *= sa
    shift = shift.copy()
    shift[0] *= sa
    shift[1] *= sa

    # stream s=0 (y->o1): q=conv1, k=conv2, v=conv4
    # stream s=1 (x->o2): q=conv0, k=conv3, v=conv5
    qk_ids = [[1, 2], [0, 3]]
    v_ids = [4, 5]

    # wqk[s, qk, pair, ci_chunk, ci, tap, (hl,co)] = wf[conv, h, co, ci, dy, dx]
    CC = C // P
    wqk = np.empty([2, 2, HEADS // 2, CC, P, 9, 2 * C], np.float32)
    # wvf[s, cich, tap, dP, h, dch, ciP] = wf[conv_v, h, dch*P+dP,
    #                                        cich*P+ciP, tap//3, tap%3]
    wvf = np.empty([2, CC, 9, P, HEADS, CC, P], np.float32)
    for s in range(2):
        for j, conv in enumerate(qk_ids[s]):
            # [pr, hl, co, ci, tap] -> [pr, ci_chunk, ci, tap, hl, co]
            t = wf[conv].reshape(HEADS // 2, 2, C, C, 9).transpose(0, 3, 4, 1, 2)
            wqk[s, j] = t.reshape(HEADS // 2, C // P, P, 9, 2 * C)
        t = wf[v_ids[s]].reshape(HEADS, CC, P, CC, P, 9)
        wvf[s] = t.transpose(3, 5, 2, 0, 1, 4)

    # bqk[s, qk, 128, h, co] = shift[conv][h, co] / 128 (replicated)
    bqk = np.empty([2, 2, P, HEADS, C], np.float32)
    for s in range(2):
        for j, conv in enumerate(qk_ids[s]):
            bqk[s, j] = np.broadcast_to(shift[conv][None], (P, HEADS, C)) / P

    # bv[s, h, dchunk, 128, 8] (replicated x8: fp32r free-dim >= 2)
    bv = np.empty([2, HEADS, CC, P, 8], np.float32)
    for s in range(2):
        bv[s] = np.repeat(
            shift[v_ids[s]].reshape(HEADS, CC, P)[..., None], 8, axis=-1)

    # wo[s, cchunk, c, co] = w_out.T / heads
    wo = np.empty([2, C // P, P, C], np.float32)
    wo[0] = (np.asarray(w_out1, np.float32).T / HEADS).reshape(C // P, P, C)
    wo[1] = (np.asarray(w_out2, np.float32).T / HEADS).reshape(C // P, P, C)

    return wqk, wvf, bqk, bv, wo


def kernel(x, y, w_conv, bn_gamma, bn_beta, bn_mean, bn_var, w_out1, w_out2,
           h, w):
    H, W = int(h), int(w)
    x = np.asarray(x, np.float32)
    y = np.asarray(y, np.float32)
    B = x.shape[0]
    assert B == NCORES, f"expected B={NCORES}, got {B}"

    key = (H, W)
    if key not in _programs:
        _programs[key] = _build_program(H, W)
    nc = _programs[key]

    wqk, wvf, bqk, bv, wo = _prep_weights(
        w_conv, bn_gamma, bn_beta, bn_mean, bn_var, w_out1, w_out2)

    in_maps = []
    for b in range(B):
        in_maps.append({
            "in0": np.ascontiguousarray(y[b]),   # stream 0: y -> o1
            "in1": np.ascontiguousarray(x[b]),   # stream 1: x -> o2
            "wqk": wqk, "wvf": wvf, "bqk": bqk, "bv": bv, "wo": wo,
        })

    trace = bool(int(os.environ.get("KERNEL_TRACE", "0")))
    res = run_bass_kernel_spmd(nc, in_maps, core_ids=list(range(NCORES)),
                               trace=trace)
    if trace:
        tr = res.instructions_and_trace
        print(f"[kernel] HW exec_time_ns={res.exec_time_ns} "
              f"mean={res.mean_exec_time_ns} "
              f"trace={tr[1] if tr else None}")
        kernel.last_exec_ns = res.exec_time_ns
        kernel.last_result = res

    o1 = np.stack([res.results[b]["out0"] for b in range(B)])
    o2 = np.stack([res.results[b]["out1"] for b in range(B)])
    return o1, o2

